# revision 7
# baseline (speedup 1.0000x reference)
"""Chamfer loss kernel for Trainium2 (8 NeuronCores, data-parallel over batch).

Math: for each batch, d2[m,n] = ||pred_m - gt_n||^2 = p2[m] + g2[n] - 2*dot.
The reference gathers the argmin point and recomputes the distance, which
equals min_n d2[m,n] (resp. min_m), so no argmin/gather is needed:
  fwd_e = sqrt(rowmin(d2) + EPS), bwd_e = sqrt(colmin(d2) + EPS)
  loss = mean(relu(fwd_e - t)) + mean(relu(bwd_e - t))

Device work per core (2 batches): d2 via K=5 fp32 matmul with augmented
operands A = [-2*pred; p2; 1] (lhsT) and B = [gt; 1; g2] (rhs); PE writes
[128,512] PSUM tiles; ACT copies PSUM->SBUF as NEGATED fp16 (s = -d2, so all
reductions are max-based); DVE does the col-max accumulation (elementwise max
across m-tiles) and row-max (binary tree at 2x fp16 rate + final reduce);
GPSIMD partition_all_reduce(max) collapses the col accumulator across
partitions.  Host does the tiny epilogue: negate, sqrt/relu/mean on 128K
values.
"""

import os
from contextlib import ExitStack

import numpy as np

EPS = 1e-8
B, M, N = 16, 4096, 4096
NCORES = 8
B_LOC = B // NCORES  # batches per core

_CACHE = {}


def build_nc(b_loc=B_LOC, m=M, n=N, reps=1):
    import concourse.bacc as bacc
    import concourse.mybir as mybir
    import concourse.tile as tile
    from concourse import bass_isa

    f32 = mybir.dt.float32
    f16 = mybir.dt.float16
    MAX = mybir.AluOpType.max
    Copy = mybir.ActivationFunctionType.Copy

    nc = bacc.Bacc("TRN2", target_bir_lowering=False, debug=False)
    a_in = nc.dram_tensor("a_in", [b_loc, 5, m], f32, kind="ExternalInput").ap()
    b_in = nc.dram_tensor("b_in", [b_loc, 5, n], f32, kind="ExternalInput").ap()
    n_mt = m // 128
    # fwd_out[b, p, mt] = max_n(-d2[mt*128+p, n]) = -rowmin
    fwd_out = nc.dram_tensor(
        "fwd_out", [b_loc, 128, n_mt], f32, kind="ExternalOutput"
    ).ap()
    # bwd_out[b, 0, n] = max_m(-d2[m, n]) = -colmin
    bwd_out = nc.dram_tensor(
        "bwd_out", [b_loc, 1, n], f32, kind="ExternalOutput"
    ).ap()

    n_half = n // 2
    with tile.TileContext(nc) as tc, ExitStack() as ctx:
        ab_pool = ctx.enter_context(tc.tile_pool(name="ab", bufs=2))
        ps_pool = ctx.enter_context(tc.tile_pool(name="ps", bufs=2, space="PSUM"))
        sb_pool = ctx.enter_context(tc.tile_pool(name="sb", bufs=3))
        w_pool = ctx.enter_context(tc.tile_pool(name="w", bufs=2))
        cacc_pool = ctx.enter_context(tc.tile_pool(name="cacc", bufs=2))
        fwd_pool = ctx.enter_context(tc.tile_pool(name="fwd", bufs=2))
        pr_pool = ctx.enter_context(tc.tile_pool(name="pr", bufs=2))

        for _ in range(reps):
            for b in range(b_loc):
                a_sb = ab_pool.tile([5, m], f32, tag="a")
                b_sb = ab_pool.tile([5, n], f32, tag="b")
                nc.sync.dma_start(out=a_sb, in_=a_in[b])
                nc.sync.dma_start(out=b_sb, in_=b_in[b])

                cacc = cacc_pool.tile([128, n], f16)
                fwd = fwd_pool.tile([128, n_mt], f32)

                for mt in range(n_mt):
                    lhsT = a_sb[:, mt * 128 : (mt + 1) * 128]
                    sb = sb_pool.tile([128, n], f16)
                    for h in range(2):
                        ps = ps_pool.tile([128, n_half], f32)
                        for j in range(n_half // 512):
                            n0 = h * n_half + j * 512
                            nc.tensor.matmul(
                                ps[:, j * 512 : (j + 1) * 512],
                                lhsT,
                                b_sb[:, n0 : n0 + 512],
                                start=True,
                                stop=True,
                            )
                        # negate on the way out of PSUM: sb = -d2 (fp16)
                        nc.scalar.activation(
                            out=sb[:, h * n_half : (h + 1) * n_half],
                            in_=ps,
                            func=Copy,
                            scale=-1.0,
                        )
                    # col-max accumulate across m-tiles (elementwise, fp16 2x)
                    if mt == 0:
                        nc.vector.tensor_copy(out=cacc, in_=sb)
                    else:
                        nc.vector.tensor_tensor(out=cacc, in0=cacc, in1=sb, op=MAX)
                    # row-max: binary max-tree along free dim (fp16 2x),
                    # final 1x reduce on the last 256 elements
                    w = sb
                    size = n
                    while size > 256:
                        size //= 2
                        wn = w_pool.tile([128, size], f16, tag=f"w{size}")
                        nc.vector.tensor_tensor(
                            out=wn, in0=w[:, 0:size], in1=w[:, size : 2 * size], op=MAX
                        )
                        w = wn
                    nc.vector.tensor_reduce(
                        out=fwd[:, mt : mt + 1],
                        in_=w,
                        axis=mybir.AxisListType.X,
                        op=MAX,
                    )

                # collapse col accumulator across partitions on GPSIMD
                pr = pr_pool.tile([128, n], f32)
                nc.gpsimd.partition_all_reduce(
                    pr, cacc, channels=128, reduce_op=bass_isa.ReduceOp.max
                )
                nc.sync.dma_start(out=fwd_out[b], in_=fwd)
                nc.sync.dma_start(out=bwd_out[b], in_=pr[0:1, :])
    nc.compile()
    return nc


def _host_prep(predict_pc_6, gt_pc_6):
    """Build augmented matmul operands A (lhsT side) and B (rhs side)."""
    pred = np.ascontiguousarray(predict_pc_6[:, :3, :], dtype=np.float32)
    gt = np.ascontiguousarray(gt_pc_6[:, :3, :], dtype=np.float32)
    A = np.empty((B, 5, M), np.float32)
    A[:, 0:3] = -2.0 * pred
    A[:, 3] = np.einsum("bdm,bdm->bm", pred, pred)
    A[:, 4] = 1.0
    Bm = np.empty((B, 5, N), np.float32)
    Bm[:, 0:3] = gt
    Bm[:, 3] = 1.0
    Bm[:, 4] = np.einsum("bdm,bdm->bm", gt, gt)
    return A, Bm


def kernel(predict_pc_6, gt_pc_6, thresh):
    from concourse.bass_utils import run_bass_kernel_spmd

    predict_pc_6 = np.asarray(predict_pc_6)
    gt_pc_6 = np.asarray(gt_pc_6)
    thresh = np.float32(thresh)

    A, Bm = _host_prep(predict_pc_6, gt_pc_6)

    if "nc" not in _CACHE:
        _CACHE["nc"] = build_nc()
    nc = _CACHE["nc"]

    core_ids = list(range(NCORES))
    in_maps = [
        {
            "a_in": np.ascontiguousarray(A[i * B_LOC : (i + 1) * B_LOC]),
            "b_in": np.ascontiguousarray(Bm[i * B_LOC : (i + 1) * B_LOC]),
        }
        for i in core_ids
    ]
    res = run_bass_kernel_spmd(nc, in_maps, core_ids)
    _CACHE["last_res"] = res

    # Host epilogue on 8 * 2 * (4096 + 4096) values.
    fwd_sum = 0.0
    bwd_sum = 0.0
    for i in core_ids:
        r = res.results[i]
        rowmin = -r["fwd_out"].astype(np.float64).reshape(-1)
        colmin = -r["bwd_out"].astype(np.float64).reshape(-1)
        fwd_e = np.sqrt(rowmin + EPS)
        bwd_e = np.sqrt(colmin + EPS)
        fwd_sum += np.maximum(fwd_e - float(thresh), 0.0).sum()
        bwd_sum += np.maximum(bwd_e - float(thresh), 0.0).sum()

    loss = fwd_sum / (B * M) + bwd_sum / (B * N)
    return np.float32(loss)


# revision 9
# speedup vs baseline: 8.4300x; 8.4300x over previous
"""Chamfer loss kernel for Trainium2 (8 NeuronCores, data-parallel over batch).

Math: for each batch, d2[m,n] = ||pred_m - gt_n||^2 = p2[m] + g2[n] - 2*dot.
The reference gathers the argmin point and recomputes the distance, which
equals min_n d2[m,n] (resp. min_m), so no argmin/gather is needed:
  fwd_e = sqrt(rowmin(d2) + EPS), bwd_e = sqrt(colmin(d2) + EPS)
  loss = mean(relu(fwd_e - t)) + mean(relu(bwd_e - t))

Device work per core (2 batches): d2 via K=5 fp32 matmul with augmented
operands A = [-2*pred; p2; 1] (lhsT) and B = [gt; 1; g2] (rhs); PE writes
[128,512] PSUM tiles; ACT copies PSUM->SBUF as NEGATED fp16 (s = -d2, so all
reductions are max-based); DVE does the col-max accumulation (elementwise max
across m-tiles) and row-max (binary tree at 2x fp16 rate + final reduce);
GPSIMD partition_all_reduce(max) collapses the col accumulator across
partitions.  Host does the tiny epilogue: negate, sqrt/relu/mean on 128K
values.
"""

import os
from contextlib import ExitStack

import numpy as np

EPS = 1e-8
B, M, N = 16, 4096, 4096
NCORES = 8
B_LOC = B // NCORES  # batches per core

_CACHE = {}


def build_nc(b_loc=B_LOC, m=M, n=N, reps=1, G=4):
    import concourse.bacc as bacc
    import concourse.mybir as mybir
    import concourse.tile as tile
    from concourse import bass_isa
    from concourse.bass import ds

    f32 = mybir.dt.float32
    f16 = mybir.dt.float16
    MAX = mybir.AluOpType.max
    Copy = mybir.ActivationFunctionType.Copy
    E = mybir.EngineType

    nc = bacc.Bacc("TRN2", target_bir_lowering=False, debug=False)
    a_in = nc.dram_tensor("a_in", [b_loc, 5, m], f32, kind="ExternalInput").ap()
    b_in = nc.dram_tensor("b_in", [b_loc, 5, n], f32, kind="ExternalInput").ap()
    n_mt = m // 128
    n_grp = n_mt // G
    # fwd_out[b, p, mt] = max_n(-d2[mt*128+p, n]) = -rowmin
    fwd_out = nc.dram_tensor(
        "fwd_out", [b_loc, 128, n_mt], f32, kind="ExternalOutput"
    ).ap()
    # bwd_out[b, 0, n] = max_m(-d2[m, n]) = -colmin
    bwd_out = nc.dram_tensor(
        "bwd_out", [b_loc, 1, n], f32, kind="ExternalOutput"
    ).ap()

    hints = (E.PE, E.Activation, E.DVE, E.SP, E.Pool)
    with tile.TileContext(nc) as tc, ExitStack() as ctx:
        ab_pool = ctx.enter_context(tc.tile_pool(name="ab", bufs=2))
        ps_pool = ctx.enter_context(tc.tile_pool(name="ps", bufs=2, space="PSUM"))
        sb_pool = ctx.enter_context(tc.tile_pool(name="sb", bufs=2))
        w_pool = ctx.enter_context(tc.tile_pool(name="w", bufs=1))
        cp = ctx.enter_context(tc.tile_pool(name="c", bufs=1))

        for _ in range(reps):
            for b in range(b_loc):
                a_sb = ab_pool.tile([5, m], f32, tag="a")
                b_sb = ab_pool.tile([5, n], f32, tag="b")
                nc.sync.dma_start(out=a_sb, in_=a_in[b])
                nc.sync.dma_start(out=b_sb, in_=b_in[b])

                cacc = cp.tile([128, n], f16, tag="cacc")
                fwd = cp.tile([128, n_mt], f32, tag="fwd")
                wcur = cp.tile([5, G * 128], f32, tag="wcur")
                nc.vector.memset(cacc, -60000.0)

                with tc.For_i(0, n_grp, 1, hint_engines=hints) as k:
                    # stage this group's G m-tiles of weights (dynamic src)
                    nc.vector.tensor_copy(
                        out=wcur, in_=a_sb[:, ds(k * (G * 128), G * 128)]
                    )
                    sb = sb_pool.tile([128, G, n], f16, tag="sb")
                    for u in range(G):
                        for h in range(2):
                            ps = ps_pool.tile([128, n // 2], f32, tag="ps")
                            for j in range(n // 2 // 512):
                                n0 = h * (n // 2) + j * 512
                                nc.tensor.matmul(
                                    ps[:, j * 512 : (j + 1) * 512],
                                    wcur[:, u * 128 : (u + 1) * 128],
                                    b_sb[:, n0 : n0 + 512],
                                    start=True,
                                    stop=True,
                                )
                            # negate on the way out of PSUM: sb = -d2 (fp16)
                            nc.scalar.activation(
                                out=sb[:, u, h * (n // 2) : (h + 1) * (n // 2)],
                                in_=ps,
                                func=Copy,
                                scale=-1.0,
                            )
                    # col-max accumulate: fold G m-tiles pairwise, then into cacc
                    t1 = w_pool.tile([128, G // 2, n], f16, tag="t1")
                    nc.vector.tensor_tensor(
                        out=t1, in0=sb[:, 0 : G // 2, :], in1=sb[:, G // 2 : G, :], op=MAX
                    )
                    fold = t1
                    gg = G // 2
                    while gg > 1:
                        gg //= 2
                        nc.vector.tensor_tensor(
                            out=fold[:, 0:gg, :],
                            in0=fold[:, 0:gg, :],
                            in1=fold[:, gg : 2 * gg, :],
                            op=MAX,
                        )
                    nc.vector.tensor_tensor(
                        out=cacc, in0=cacc, in1=fold[:, 0, :], op=MAX
                    )
                    # row-max: batched binary tree across all G m-tiles
                    w = sb
                    size = n
                    while size > 256:
                        size //= 2
                        wn = w_pool.tile([128, G, size], f16, tag=f"w{size}")
                        nc.vector.tensor_tensor(
                            out=wn, in0=w[:, :, 0:size], in1=w[:, :, size : 2 * size],
                            op=MAX,
                        )
                        w = wn
                    nc.vector.tensor_reduce(
                        out=fwd[:, ds(k * G, G)],
                        in_=w,
                        axis=mybir.AxisListType.X,
                        op=MAX,
                    )

                # collapse col accumulator across partitions on GPSIMD
                pr = cp.tile([128, n], f32, tag="pr")
                nc.gpsimd.partition_all_reduce(
                    pr, cacc, channels=128, reduce_op=bass_isa.ReduceOp.max
                )
                nc.sync.dma_start(out=fwd_out[b], in_=fwd)
                nc.sync.dma_start(out=bwd_out[b], in_=pr[0:1, :])
    nc.compile()
    return nc


def _host_prep(predict_pc_6, gt_pc_6):
    """Build augmented matmul operands A (lhsT side) and B (rhs side)."""
    pred = np.ascontiguousarray(predict_pc_6[:, :3, :], dtype=np.float32)
    gt = np.ascontiguousarray(gt_pc_6[:, :3, :], dtype=np.float32)
    A = np.empty((B, 5, M), np.float32)
    A[:, 0:3] = -2.0 * pred
    A[:, 3] = np.einsum("bdm,bdm->bm", pred, pred)
    A[:, 4] = 1.0
    Bm = np.empty((B, 5, N), np.float32)
    Bm[:, 0:3] = gt
    Bm[:, 3] = 1.0
    Bm[:, 4] = np.einsum("bdm,bdm->bm", gt, gt)
    return A, Bm


def kernel(predict_pc_6, gt_pc_6, thresh):
    from concourse.bass_utils import run_bass_kernel_spmd

    predict_pc_6 = np.asarray(predict_pc_6)
    gt_pc_6 = np.asarray(gt_pc_6)
    thresh = np.float32(thresh)

    A, Bm = _host_prep(predict_pc_6, gt_pc_6)

    if "nc" not in _CACHE:
        _CACHE["nc"] = build_nc()
    nc = _CACHE["nc"]

    core_ids = list(range(NCORES))
    in_maps = [
        {
            "a_in": np.ascontiguousarray(A[i * B_LOC : (i + 1) * B_LOC]),
            "b_in": np.ascontiguousarray(Bm[i * B_LOC : (i + 1) * B_LOC]),
        }
        for i in core_ids
    ]
    res = run_bass_kernel_spmd(nc, in_maps, core_ids)
    _CACHE["last_res"] = res

    # Host epilogue on 8 * 2 * (4096 + 4096) values.
    fwd_sum = 0.0
    bwd_sum = 0.0
    for i in core_ids:
        r = res.results[i]
        rowmin = -r["fwd_out"].astype(np.float64).reshape(-1)
        colmin = -r["bwd_out"].astype(np.float64).reshape(-1)
        fwd_e = np.sqrt(rowmin + EPS)
        bwd_e = np.sqrt(colmin + EPS)
        fwd_sum += np.maximum(fwd_e - float(thresh), 0.0).sum()
        bwd_sum += np.maximum(bwd_e - float(thresh), 0.0).sum()

    loss = fwd_sum / (B * M) + bwd_sum / (B * N)
    return np.float32(loss)
